# revision 19
# baseline (speedup 1.0000x reference)
"""Trainium2 Bass kernel for nn_CMAE_8856222564944 (retrieval_knn).

Computation (reference):
    h = L2-normalize rows of x            [B, N_ITEMS]
    h = tanh(h @ W1 + b1)                 [B, 600]
    h = tanh(h @ W2 + b2)                 [B, 200]
    h = tanh(h @ W3 + b3)                 [B, 600]
    dist = |h|^2 - 2 h @ E^T + |E|^2      [B, N_ITEMS]

Distribution (8 cores, tensor-parallel over the item dim):
    - x^T, W1, E^T are sharded over items (rows of W1/x^T, cols of E^T).
    - Each core computes a partial u^T = W1_sh^T x_sh^T; one AllReduce of
      the small [600, B] hidden; the W2/W3 layers are replicated.
    - Each core computes its column shard of dist and the host concatenates.

Precision: the two big GEMMs (phase-1 x@W1 and dist h@E^T) run in fp8-e4m3
with perf_mode=DoubleRow (256-row contraction per matmul).  Scales keep
values in fp8's sweet spot:
    x*128, W1*32  -> psum = 4096*u, descaled inside the tanh activation.
    h*32, (-2E)*16 -> psum = 512*(-2 h.E), descaled in the output copy.
The |h|^2 / |E|^2 epilogue rides as augmented fp8 contraction rows:
    row600: (32*h_sq_hi)        x 16
    row601: (32*h_sq - hi)*64   x 0.25        (hi/lo split => ~exact h_sq)
    row602: 1                   x 512*(e_sq - 0.25)
    row603: 16                  x 8           (= 512*0.25, restores the mean)
The mid MLP (tiny) stays bf16; dist ships back as bf16.
"""

import sys

if "/opt/trn_rl_repo" not in sys.path:
    sys.path.insert(0, "/opt/trn_rl_repo")

import numpy as np
import ml_dtypes

import concourse.bass as bass
import concourse.mybir as mybir
import concourse.tile as tile
from concourse import bacc

BF16 = ml_dtypes.bfloat16
FP8 = ml_dtypes.float8_e4m3  # TRN FP8_EXP4: max normal +-240
P = 128

# Full-size problem config
N_CORES = 8
B = 1024
H1 = 600
H1P = 608                  # fp8 DoubleRow needs 16B-aligned k-subtile step
H2 = 200
N_ITEMS = 50000
ITEMS_PAD = 50176          # 8 * 6272, 6272 = 49 * 128
SH = ITEMS_PAD // N_CORES  # per-core item shard
KD = 604                   # dist contraction rows: 600 h + 4 aug

# fp8 scales
SX = 128.0                 # x (unit-norm rows, entries ~0.004)
SW1 = 32.0                 # W1 (~N(0, 0.02^2))
SU = SX * SW1              # 4096: phase-1 psum descale
SHS = 32.0                 # h   (~N(0, 0.017^2))
SES = 16.0                 # -2E (~N(0, 0.04^2))
SD = SHS * SES             # 512: dist psum descale


def _chunks(total, size):
    """[(start, length), ...] covering [0, total) in `size` steps."""
    return [(s, min(size, total - s)) for s in range(0, total, size)]




def _dedup_ldweights(nc):
    """Post-scheduling: delete LDWEIGHTS whose stationary operand is identical
    to the previous LDWEIGHTS in the final PE stream (the array still holds
    those weights). Waits carried by a deleted load transfer to the next PE
    instruction so no dependency is lost."""
    import concourse.mybir as mb

    n_skipped = 0
    for bb in nc.main_func.blocks:
        insts = bb.instructions
        prev_key = None
        kill = {}
        for idx, ins in enumerate(insts):
            if getattr(ins, "engine", None) != mb.EngineType.PE:
                continue
            if isinstance(ins, mb.InstLdweights):
                key = (
                    str(ins.ins[0]),
                    ins.perf_mode,
                    ins.is_transpose,
                    ins.tile_position,
                    ins.tile_size,
                )
                if key == prev_key:
                    kill[idx] = ins
                else:
                    prev_key = key
            elif isinstance(ins, (mb.InstMatmult, mb.InstEventSemaphore, mb.InstNoOp)):
                pass  # these leave the loaded weights intact
            else:
                prev_key = None
        if not kill:
            continue
        new_insts = []
        pending = []
        for idx, ins in enumerate(insts):
            if idx in kill:
                pending.append(ins)
                continue
            if pending and getattr(ins, "engine", None) == mb.EngineType.PE:
                for dead in pending:
                    ins.merge_dependencies_from(dead)
                pending = []
            new_insts.append(ins)
        assert not pending
        bb.instructions = new_insts
        n_skipped += len(kill)
    print(f"_dedup_ldweights: removed {n_skipped} redundant weight loads")


def build_program(b=B, h1=H1, h2=H2, sh=SH, n_cores=N_CORES):
    """Build the per-core SPMD Bass program (same graph on every core)."""
    dt = mybir.dt
    fp32 = dt.float32
    bf16 = dt.bfloat16
    fp8 = dt.float8e4

    assert sh % P == 0
    k1 = sh // P                      # 128-row item K-subtiles for phase 1
    kp1 = (k1 + 1) // 2               # DoubleRow pair iterations (24 DR + 1)
    mch = _chunks(h1, P)              # H1 row subtiles: [(0,128)..(512,88)]
    m2ch = _chunks(h2, P)             # H2 row subtiles: [(0,128),(128,72)]
    bch = _chunks(b, 512)             # B column halves (psum free dim)
    nch = _chunks(sh, 448)            # dist output column tiles
    # dist K-subtiles: 4 full + (88 h rows + 4 aug rows)
    kdch = list(mch[:-1]) + [(mch[-1][0], mch[-1][1] + 4)]
    assert kdch[-1][1] <= P
    DR = mybir.MatmulPerfMode.DoubleRow

    nc = bacc.Bacc(
        "TRN2",
        target_bir_lowering=False,
        debug=False,
        enable_asserts=False,
        num_devices=n_cores,
    )

    xT = nc.dram_tensor("xT", [sh, b], fp8, kind="ExternalInput")
    W1d = nc.dram_tensor("W1s", [sh, H1P], fp8, kind="ExternalInput")
    W2d = nc.dram_tensor("W2s", [h1, h2], bf16, kind="ExternalInput")
    W3d = nc.dram_tensor("W3s", [h2, h1], bf16, kind="ExternalInput")
    b1d = nc.dram_tensor("b1", [h1], fp32, kind="ExternalInput")
    b2d = nc.dram_tensor("b2", [h2], fp32, kind="ExternalInput")
    b3d = nc.dram_tensor("b3", [h1], fp32, kind="ExternalInput")
    eTd = nc.dram_tensor("eT", [KD, sh], fp8, kind="ExternalInput")
    outd = nc.dram_tensor("dist", [b, sh], bf16, kind="ExternalOutput")

    Tanh = mybir.ActivationFunctionType.Tanh
    Square = mybir.ActivationFunctionType.Square
    rg = [list(range(n_cores))]

    with tile.TileContext(nc) as tc:
        with (
            tc.tile_pool(name="persist", bufs=1) as persist,
            tc.tile_pool(name="dram", bufs=1, space="DRAM") as dram,
            tc.tile_pool(name="psum", bufs=1, space="PSUM") as psum_pool,
            tc.tile_pool(name="outs", bufs=6) as out_pool,
        ):
            # ---- persistent SBUF tensors -------------------------------
            x_sb = persist.tile([P, k1, b], fp8, name="x_sb")
            W1_sb = persist.tile([P, k1, H1P], fp8, name="W1_sb")
            e_sb = persist.tile([P, len(kdch), sh], fp8, name="e_sb")
            W2_sb = persist.tile([P, len(mch), h2], bf16, name="W2_sb")
            W3_sb = persist.tile([P, len(m2ch), h1], bf16, name="W3_sb")
            b1_sb = persist.tile([P, len(mch), 1], fp32, name="b1_sb")
            b2_sb = persist.tile([P, len(m2ch), 1], fp32, name="b2_sb")
            b3_sb = persist.tile([P, len(mch), 1], fp32, name="b3_sb")
            ones_sb = persist.tile([P, len(mch), 1], bf16, name="ones_sb")
            uq_sb = persist.tile([P, len(mch), b], fp8, name="uq_sb")
            up_sb = persist.tile([P, len(mch), b], bf16, name="up_sb")
            h1_sb = persist.tile([P, len(mch), b], bf16, name="h1_sb")
            h2_sb = persist.tile([P, len(m2ch), b], bf16, name="h2_sb")
            hhat_sb = persist.tile([P, len(kdch), b], fp8, name="hhat_sb")
            hq32_sb = persist.tile([1, b], fp32, name="hq32_sb")
            hqhi_sb = persist.tile([1, b], fp8, name="hqhi_sb")
            hqr_sb = persist.tile([1, b], fp32, name="hqr_sb")
            hqlo_sb = persist.tile([1, b], fp8, name="hqlo_sb")

            c1_sb = persist.tile([1, b], fp8, name="c1_sb")
            c16_sb = persist.tile([1, b], fp8, name="c16_sb")

            nc.vector.memset(ones_sb[:], 1.0)
            # h-side const aug rows (pair with e-side e_sq rows); compute
            # engines can't address partition 90/91 directly -> DMA them in
            nc.vector.memset(c1_sb[:], 1.0)
            nc.vector.memset(c16_sb[:], 16.0)
            nc.scalar.dma_start(hhat_sb[90:91, len(kdch) - 1, :], c1_sb[0:1, :])
            nc.scalar.dma_start(hhat_sb[91:92, len(kdch) - 1, :], c16_sb[0:1, :])
            nc.scalar.activation(hq32_sb[0:1, 0:1], ones_sb[0:1, 0, 0:1], Tanh)

            # ---- phase 1: partial u^T = W1_sh^T @ x_sh^T ----------------
            u_bounce = []
            u_red = []
            for hi, (c0, cl) in enumerate(bch):
                u_bounce.append(
                    dram.tile([h1, cl], fp8, name=f"u_bounce{hi}")
                )
                u_red.append(
                    dram.tile(
                        [h1, cl],
                        fp8,
                        addr_space="Shared" if n_cores > 4 else "Local",
                        name=f"u_red{hi}",
                    )
                )

            # upfront streaming loads: x (sync ring) and W1 then e^T (scalar
            # ring) — all contiguous 128-row blocks with wide DMA lines; the
            # bulk traffic drains before the first AllReduce's window so the
            # collective's own DMAs don't contend
            for k in range(k1):
                nc.sync.dma_start(x_sb[:, k, :], xT[k * P : (k + 1) * P, :])
                nc.scalar.dma_start(W1_sb[:, k, :], W1d[k * P : (k + 1) * P, :])
            # small constants on the gpsimd ring (drains in ~1us, well
            # before the u bounce DMAs join it) — needed by tanh as soon as
            # the first AllReduce lands
            for ki, (m0, ml) in enumerate(mch):
                nc.gpsimd.dma_start(
                    b1_sb[:ml, ki, :], b1d[m0 : m0 + ml].rearrange("(p o) -> p o", o=1)
                )
                nc.gpsimd.dma_start(
                    b3_sb[:ml, ki, :], b3d[m0 : m0 + ml].rearrange("(p o) -> p o", o=1)
                )
                nc.gpsimd.dma_start(W2_sb[:ml, ki, :], W2d[m0 : m0 + ml, :])
            for ki, (m0, ml) in enumerate(m2ch):
                nc.gpsimd.dma_start(
                    b2_sb[:ml, ki, :], b2d[m0 : m0 + ml].rearrange("(p o) -> p o", o=1)
                )
                nc.gpsimd.dma_start(W3_sb[:ml, ki, :], W3d[m0 : m0 + ml, :])

            for hi, (c0, cl) in enumerate(bch):
                psums = [
                    psum_pool.tile([P, 512], fp32, name=f"p1_{hi}_{mi}", tag=f"pbank{mi}")
                    for mi in range(len(mch))
                ]
                for j in range(kp1):
                    pair = 2 * j + 1 < k1
                    for mi, (m0, ml) in enumerate(mch):
                        if pair:
                            nc.tensor.matmul(
                                psums[mi][:ml, :cl],
                                W1_sb[:, 2 * j : 2 * j + 2, m0 : m0 + ml],
                                x_sb[:, 2 * j : 2 * j + 2, c0 : c0 + cl],
                                start=(j == 0),
                                stop=(j == kp1 - 1),
                                perf_mode=DR,
                            )
                        else:
                            nc.tensor.matmul(
                                psums[mi][:ml, :cl],
                                W1_sb[:, 2 * j, m0 : m0 + ml],
                                x_sb[:, 2 * j, c0 : c0 + cl],
                                start=(j == 0),
                                stop=(j == kp1 - 1),
                            )
                # bounce the partial u out in fp8 (halves collective bytes);
                # the gpsimd ring holds only these DMAs + the trigger, so the
                # collective fires as soon as the last line lands
                for mi, (m0, ml) in enumerate(mch):
                    # descale 4096u -> 128u so the fp8 wire format can't
                    # saturate (sum sigma ~2.6, fp8 max 240); alternate
                    # engines so the five copies drain in parallel
                    if mi % 2 == 0:
                        nc.scalar.mul(uq_sb[:ml, mi, c0 : c0 + cl],
                                      psums[mi][:ml, :cl], 1.0 / 32.0)
                    else:
                        nc.vector.tensor_scalar_mul(uq_sb[:ml, mi, c0 : c0 + cl],
                                                    psums[mi][:ml, :cl], 1.0 / 32.0)
                    nc.gpsimd.dma_start(
                        u_bounce[hi][m0 : m0 + ml, :], uq_sb[:ml, mi, c0 : c0 + cl]
                    )
                nc.gpsimd.collective_compute(
                    "AllReduce",
                    mybir.AluOpType.add,
                    replica_groups=rg,
                    ins=[u_bounce[hi].opt()],
                    outs=[u_red[hi].opt()],
                )
                if hi == 0:
                    # e^T preload: the scalar engine reaches these dma_starts
                    # only after the h0 psum copies, so the 3.8MB e stream
                    # never competes with the x/W1 streams feeding phase 1
                    for ki_ in range(len(kdch)):
                        r0_, rl_ = kdch[ki_]
                        for ec0, ecl in _chunks(sh, sh // 4):
                            nc.scalar.dma_start(
                                e_sb[:rl_, ki_, ec0 : ec0 + ecl],
                                eTd[r0_ : r0_ + rl_, ec0 : ec0 + ecl],
                            )

            # ---- per-B-half tail: tanh -> W2 -> W3 -> h_sq -> dist ------
            last_k = len(kdch) - 1
            hrow = mch[-1][1]          # first aug partition in last subtile
            group_sz = 3
            ngroups = [nch[i : i + group_sz] for i in range(0, len(nch), group_sz)]

            def emit_tanh(hi):
                c0, cl = bch[hi]
                for mi, (m0, ml) in enumerate(mch):
                    nc.scalar.dma_start(
                        uq_sb[:ml, mi, c0 : c0 + cl], u_red[hi][m0 : m0 + ml, :]
                    )
                    nc.scalar.activation(
                        h1_sb[:ml, mi, c0 : c0 + cl],
                        uq_sb[:ml, mi, c0 : c0 + cl],
                        Tanh,
                        bias=b1_sb[:ml, mi, 0:1],
                        scale=32.0 / SU,
                    )

            def emit_mlp(hi):
                c0, cl = bch[hi]
                # phase 2 (pbank6/7 — never used by dist)
                for mi, (m0, ml) in enumerate(m2ch):
                    ps = psum_pool.tile([P, 512], fp32, name=f"p2_{hi}_{mi}", tag=f"pbank{6 + mi}")
                    for k, (r0, rl) in enumerate(mch):
                        nc.tensor.matmul(
                            ps[:ml, :cl],
                            W2_sb[:rl, k, m0 : m0 + ml],
                            h1_sb[:rl, k, c0 : c0 + cl],
                            start=(k == 0),
                            stop=(k == len(mch) - 1),
                        )
                    nc.scalar.activation(
                        h2_sb[:ml, mi, c0 : c0 + cl],
                        ps[:ml, :cl],
                        Tanh,
                        bias=b2_sb[:ml, mi, 0:1],
                    )
                # phase 3 (alternates pbank6/7); tanh -> bf16 (up_sb reused),
                # then DVE cast *32 -> fp8 hhat
                for mi, (m0, ml) in enumerate(mch):
                    ps = psum_pool.tile([P, 512], fp32, name=f"p3_{hi}_{mi}", tag=f"pbank{6 + mi % 2}")
                    for k, (r0, rl) in enumerate(m2ch):
                        nc.tensor.matmul(
                            ps[:ml, :cl],
                            W3_sb[:rl, k, m0 : m0 + ml],
                            h2_sb[:rl, k, c0 : c0 + cl],
                            start=(k == 0),
                            stop=(k == len(m2ch) - 1),
                        )
                    nc.scalar.activation(
                        up_sb[:ml, mi, c0 : c0 + cl],
                        ps[:ml, :cl],
                        Tanh,
                        bias=b3_sb[:ml, mi, 0:1],
                    )
                    nc.vector.tensor_scalar_mul(
                        hhat_sb[:ml, mi, c0 : c0 + cl],
                        up_sb[:ml, mi, c0 : c0 + cl],
                        SHS,
                    )
                # h_sq = column sums of h^2 (bf16 tanh output squared on the
                # scalar engine right after each tanh; keeps the aug-row
                # chain off the DVE cast's tail)
                for ki, (m0, ml) in enumerate(mch):
                    nc.vector.tensor_mul(
                        h1_sb[:ml, ki, c0 : c0 + cl],
                        up_sb[:ml, ki, c0 : c0 + cl],
                        up_sb[:ml, ki, c0 : c0 + cl],
                    )
                psq = psum_pool.tile([1, 512], fp32, name=f"pq_{hi}", tag="pbank6")
                for k, (m0, ml) in enumerate(mch):
                    nc.tensor.matmul(
                        psq[:1, :cl],
                        ones_sb[:ml, k, 0:1],
                        h1_sb[:ml, k, c0 : c0 + cl],
                        start=(k == 0),
                        stop=(k == len(mch) - 1),
                    )
                # psq = h_sq; aug rows need 32*h_sq split hi/lo in fp8
                nc.scalar.mul(hq32_sb[0:1, c0 : c0 + cl], psq[:1, :cl], SHS)
                nc.vector.tensor_copy(
                    hqhi_sb[0:1, c0 : c0 + cl], hq32_sb[0:1, c0 : c0 + cl]
                )
                nc.vector.tensor_sub(
                    hqr_sb[0:1, c0 : c0 + cl],
                    hq32_sb[0:1, c0 : c0 + cl],
                    hqhi_sb[0:1, c0 : c0 + cl],
                )
                nc.vector.tensor_scalar_mul(
                    hqlo_sb[0:1, c0 : c0 + cl], hqr_sb[0:1, c0 : c0 + cl], 64.0
                )
                # aug rows (partitions 88/89 need DMA, not compute engines)
                nc.scalar.dma_start(
                    hhat_sb[hrow : hrow + 1, last_k, c0 : c0 + cl],
                    hqhi_sb[0:1, c0 : c0 + cl],
                )
                nc.scalar.dma_start(
                    hhat_sb[hrow + 1 : hrow + 2, last_k, c0 : c0 + cl],
                    hqlo_sb[0:1, c0 : c0 + cl],
                )

            def emit_dist(mi_list):
                kl = kdch[last_k][1]   # 92 rows in the last (non-DR) subtile
                for mi in mi_list:
                    for gi, grp in enumerate(ngroups):
                        pss = [
                            psum_pool.tile(
                                [P, 512], fp32, name=f"p4_{mi}_{gi}_{j}",
                                tag=f"pbank{(gi % 2) * 3 + j}",
                            )
                            for j in range(len(grp))
                        ]
                        for kk in range(0, last_k, 2):
                            for j, (n0, nl) in enumerate(grp):
                                nc.tensor.matmul(
                                    pss[j][:P, :nl],
                                    hhat_sb[:, kk : kk + 2, mi * P : (mi + 1) * P],
                                    e_sb[:, kk : kk + 2, n0 : n0 + nl],
                                    start=(kk == 0),
                                    stop=False,
                                    perf_mode=DR,
                                )
                        for j, (n0, nl) in enumerate(grp):
                            nc.tensor.matmul(
                                pss[j][:P, :nl],
                                hhat_sb[:kl, last_k, mi * P : (mi + 1) * P],
                                e_sb[:kl, last_k, n0 : n0 + nl],
                                start=False,
                                stop=True,
                            )
                        for j, (n0, nl) in enumerate(grp):
                            ot = out_pool.tile([P, 448], bf16, name=f"ot_{mi}_{gi}_{j}", tag="ot")
                            nc.vector.tensor_scalar_mul(
                                ot[:, :nl], pss[j][:P, :nl], 1.0 / SD
                            )
                            nc.sync.dma_start(
                                outd[mi * P : (mi + 1) * P, n0 : n0 + nl], ot[:, :nl]
                            )

            # dist(h0) runs between the two MLP halves so the second
            # AllReduce (and its tanh/MLP chain) hides under dist PE work
            half_m = [list(range(c0 // P, (c0 + cl) // P)) for c0, cl in bch]
            for hi in range(len(bch)):
                emit_tanh(hi)
                emit_mlp(hi)
                emit_dist(half_m[hi])

    _dedup_ldweights(nc)
    nc.compile()
    return nc


# ---------------------------------------------------------------------------
# Host side
# ---------------------------------------------------------------------------

def prep_inputs(x, W1, b1, W2, b2, W3, b3, item_emb, n_cores=N_CORES,
                items_pad=ITEMS_PAD):
    """Normalize/scale/cast-to-fp8/transpose/pad/shard the full inputs."""
    n_items = x.shape[1]
    b = x.shape[0]
    h1 = W1.shape[1]
    sh = items_pad // n_cores

    x = np.asarray(x, np.float32)
    norm = np.sqrt((x * x).sum(axis=1, keepdims=True))
    xn = x / np.maximum(norm, 1e-12)

    xT = np.zeros((items_pad, b), dtype=FP8)
    xT[:n_items] = (xn.T * SX).astype(FP8)
    W1p = np.zeros((items_pad, H1P), dtype=FP8)
    W1p[:n_items, :h1] = (np.asarray(W1, np.float32) * SW1).astype(FP8)

    E = np.asarray(item_emb, np.float32)
    eq = (E * (-2.0 * SES)).astype(FP8)           # [N, 600] = -32*E in fp8
    E_hat = eq.astype(np.float32) / (-2.0 * SES)  # dequantized E as device sees it
    e_sq = (E_hat * E_hat).sum(axis=1)            # consistent |E|^2
    eT = np.zeros((KD, items_pad), dtype=FP8)
    eT[:h1, :n_items] = eq.T
    eT[h1, :] = FP8(SES)                          # pairs h_sq_hi
    eT[h1 + 1, :] = FP8(0.25)                     # pairs h_sq_lo (x64 scale)
    eT[h1 + 2, :n_items] = (SD * (e_sq - 0.25)).astype(FP8)
    eT[h1 + 3, :] = FP8(8.0)                      # 16*8 = 512*0.25 mean term

    common = {
        "W2s": np.ascontiguousarray(np.asarray(W2, np.float32).astype(BF16)),
        "W3s": np.ascontiguousarray(np.asarray(W3, np.float32).astype(BF16)),
        "b1": np.asarray(b1, np.float32),
        "b2": np.asarray(b2, np.float32),
        "b3": np.asarray(b3, np.float32),
    }
    in_maps = []
    for c in range(n_cores):
        in_maps.append(
            dict(
                common,
                xT=np.ascontiguousarray(xT[c * sh : (c + 1) * sh]),
                W1s=np.ascontiguousarray(W1p[c * sh : (c + 1) * sh]),
                eT=np.ascontiguousarray(eT[:, c * sh : (c + 1) * sh]),
            )
        )
    return in_maps


_NC_CACHE = {}


def get_nc():
    if "nc" not in _NC_CACHE:
        _NC_CACHE["nc"] = build_program()
    return _NC_CACHE["nc"]


def kernel(x, W1, b1, W2, b2, W3, b3, item_emb, **run_kwargs):
    from concourse.bass_utils import run_bass_kernel_spmd

    n_items = x.shape[1]
    in_maps = prep_inputs(x, W1, b1, W2, b2, W3, b3, item_emb)
    nc = get_nc()
    res = run_bass_kernel_spmd(nc, in_maps, core_ids=list(range(N_CORES)), **run_kwargs)
    dist = np.concatenate(
        [res.results[c]["dist"] for c in range(N_CORES)], axis=1
    )[:, :n_items]
    if run_kwargs:
        kernel.last_results = res
    return np.ascontiguousarray(dist.astype(np.float32))
